# revision 11
# baseline (speedup 1.0000x reference)
"""CRF NLL loss on 8 NeuronCores — v3: scan-centric schedule.

Same math as v2 (fp8 stationary weights E'=exp(trans), kappa folded into
emissions, batched strip capture). v3:
 - feats streaming (exp), emission-gold dot and gold pair matmuls are
   INTERLEAVED into the scan loop to fill engine idle time; the scan chain
   owns only the per-step DVE pointwise.
 - ALL one-hots are host-encoded fp8 (GPSIMD is_equal is ~15.6us per
   [128,1024] op on the Q7 software engine — it backpressured the whole
   pipeline in v2). Emission-gold dot = small DVE mult+reduce slices.
 - flat 2D access patterns on the scan-critical DVE op.
 - masked readout via a 16-element indirect gather from the stop-row
   history instead of a transposed (fragmented) 32KB DMA + [16,512] Ln.
"""

import numpy as np

import concourse.bass as bass
import concourse.tile as tile
from concourse import bacc, mybir
from concourse.bass_utils import run_bass_kernel_spmd

F32 = mybir.dt.float32
BF16 = mybir.dt.bfloat16
F8 = mybir.dt.float8e4
I32 = mybir.dt.int32
I16 = mybir.dt.int16
AF = mybir.ActivationFunctionType
OP = mybir.AluOpType

B, S, T = 128, 512, 256
NCORES = 8
BL = B // NCORES
START, STOP = T - 2, T - 1
KAPPA = float(np.log(T) + 0.5)
NJ = (BL * S) // 128

F8NP = mybir.dt.np(F8)


def build_program(s_steps=S, chunk=64, kcap=None, f8=True, gold_inline=True,
                  phase=1, strip_bufs=None, pool_mult=True, do_compile=True):
    if kcap is None:
        kcap = 32
    if strip_bufs is None:
        strip_bufs = 2 if phase <= 2 else 1
    nc = bacc.Bacc("TRN2", target_bir_lowering=False, debug=False,
                   num_devices=NCORES)

    feats_sm = nc.dram_tensor("feats_sm", [T, s_steps, BL], F32, kind="ExternalInput")
    trans = nc.dram_tensor("trans", [T, T], F32, kind="ExternalInput")
    tags_bm = nc.dram_tensor("tags_bm", [BL, s_steps], I32, kind="ExternalInput")
    mask_bm = nc.dram_tensor("mask_bm", [BL, s_steps], I32, kind="ExternalInput")
    nj = (BL * s_steps) // 128
    ohprev = nc.dram_tensor("ohprev", [128, nj * T], F8, kind="ExternalInput")
    ohcur = nc.dram_tensor("ohcur", [128, nj * T], F8, kind="ExternalInput")
    nsl = s_steps * BL
    ohemA_d = nc.dram_tensor("ohemA", [128, nsl], F8, kind="ExternalInput")
    ohemB_d = nc.dram_tensor("ohemB", [128, nsl], F8, kind="ExternalInput")
    # host-precomputed index metadata (like prevT in v1): end-tag ids and
    # stop-row gather offsets (len_b-1)*BL + b
    endid_d = nc.dram_tensor("endid", [BL, 1], I32, kind="ExternalInput")
    hoff_d = nc.dram_tensor("hoff", [BL, 1], I32, kind="ExternalInput")
    out = nc.dram_tensor("out", [1, 1], F32, kind="ExternalOutput")

    h_d = nc.dram_tensor("h_d", [s_steps * BL, 1], F32)   # stop-row history (t,b)

    nch = (s_steps + chunk - 1) // chunk
    cw = chunk * BL                     # free width of one half-chunk

    with tile.TileContext(nc) as tc:
        with (
            tc.tile_pool(name="persist", bufs=1) as pp,
            tc.tile_pool(name="raw", bufs=6) as rawp,
            tc.tile_pool(name="scr", bufs=2) as scrp,
            tc.tile_pool(name="w", bufs=2) as wp,
            tc.tile_pool(name="small", bufs=1) as sp,
            tc.tile_pool(name="ps_strip", bufs=strip_bufs, space="PSUM") as ps_strip,
            tc.tile_pool(name="ps_g", bufs=1, space="PSUM") as ps_g,
            tc.tile_pool(name="ps_fin", bufs=1, space="PSUM") as ps_fin,
        ):
            # ---------------- feats chunk 0/1 DMA first (gates the scan) ----
            raws = {}

            def unit_dma(c, half):
                t0 = c * chunk
                t1c = min(s_steps, t0 + chunk)
                w = (t1c - t0) * BL
                raw = rawp.tile([128, cw], F32, tag="raw")
                nc.sync.dma_start(
                    out=raw[:, 0:w],
                    in_=feats_sm[half * 128:(half + 1) * 128, t0:t1c, :]
                    .rearrange("p a b -> p (a b)"))
                raws[(c, half)] = raw

            # mini-DMA for steps 0..7 only: the scan can start as soon as
            # these ~64KB land, instead of waiting for the full first chunk
            minis = []
            for half in (0, 1):
                rm = rawp.tile([128, 8 * BL], F32, tag="rawm")
                nc.sync.dma_start(
                    out=rm[:],
                    in_=feats_sm[half * 128:(half + 1) * 128, 0:8, :]
                    .rearrange("p a b -> p (a b)"))
                minis.append(rm)

            # ---------------- constants / transition prep ----------------
            # small DMAs on the scan-start critical path (trA -> EA -> w0)
            # are issued BEFORE the bulk chunk-0 transfer
            trA = pp.tile([128, T], F32, tag="trA")
            trB = pp.tile([128, T], F32, tag="trB")
            nc.sync.dma_start(out=trA[:], in_=trans[0:128, :])
            nc.sync.dma_start(out=trB[:], in_=trans[128:256, :])
            WD = F8 if f8 else BF16
            EA = pp.tile([128, T], WD, tag="EA")
            EB = pp.tile([128, T], WD, tag="EB")
            biasK = sp.tile([128, 1], F32, tag="biasK")
            nc.gpsimd.memset(biasK[:], -KAPPA)
            if f8:
                nc.scalar.activation(EA[:], trA[:], AF.Exp)
                nc.scalar.activation(EB[:], trB[:], AF.Exp)
            else:
                nc.scalar.activation(EA[:], trA[:], AF.Exp, bias=biasK[:])
                nc.scalar.activation(EB[:], trB[:], AF.Exp, bias=biasK[:])

            stA = sp.tile([128, 1], F32, tag="stA")
            stB = sp.tile([128, 1], F32, tag="stB")
            nc.sync.dma_start(out=stA[:], in_=trans[START:START + 1, 0:128])
            nc.sync.dma_start(out=stB[:], in_=trans[START:START + 1, 128:256])
            estA = sp.tile([128, 1], F32, tag="estA")
            estB = sp.tile([128, 1], F32, tag="estB")
            nc.scalar.activation(estA[:], stA[:], AF.Exp)
            nc.scalar.activation(estB[:], stB[:], AF.Exp)

            # bulk chunk-0 transfer after the critical small DMAs
            for half in (0, 1):
                unit_dma(0, half)

            # ---------------- tags / mask prep ----------------
            mk = sp.tile([BL, s_steps], I32, tag="mk")
            nc.sync.dma_start(out=mk[:], in_=mask_bm[:])
            maskf = sp.tile([BL, s_steps], F32, tag="maskf")
            nc.vector.tensor_copy(maskf[:], mk[:])
            endid_t = sp.tile([BL, 1], I32, tag="endid_t")
            nc.sync.dma_start(out=endid_t[:], in_=endid_d[:])
            hoff_t = sp.tile([BL, 1], I32, tag="hoff_t")
            nc.sync.dma_start(out=hoff_t[:], in_=hoff_d[:])

            unit_dma(1, 0)
            unit_dma(1, 1)

            # one-hot tiles (host-encoded fp8)
            ohpv_s = pp.tile([128, nj * T], F8, tag="ohpv_s")
            ohc_s = pp.tile([128, nj * T], F8, tag="ohc_s")
            ohemA = pp.tile([128, nsl], F8, tag="ohemA")
            ohemB = pp.tile([128, nsl], F8, tag="ohemB")

            # ---------------- streaming units (emitted interleaved) --------
            # emissions: (p, t, g, h, w): chain g's step slice is flat [128, 2*bw]
            bw = BL // phase
            gw2 = 2 * bw
            expAB = pp.tile([128, 2 * nsl], BF16, tag="expAB")
            expABv = expAB[:].rearrange("p (t g h w) -> p t g h w",
                                        g=phase, h=2, w=bw)
            ep = sp.tile([128, 8 * nch], F32, tag="ep")   # emission-gold partials

            def unit_exp(c, half):
                t0 = c * chunk
                t1c = min(s_steps, t0 + chunk)
                raw = raws[(c, half)]
                lo = 0
                if c == 0:
                    lo = 8 * BL          # steps 0..7 come from the mini tiles
                    t0 = 8
                w = t1c * BL - c * chunk * BL - lo
                nc.scalar.activation(
                    expABv[:, t0:t1c, :, half, :],
                    raw[:, lo:lo + w].rearrange("p (a g v) -> p a g v",
                                                g=phase, v=bw),
                    AF.Exp, bias=biasK[:])

            def unit_exp_mini(half):
                nc.scalar.activation(
                    expABv[:, 0:8, :, half, :],
                    minis[half][:].rearrange("p (a g v) -> p a g v",
                                             g=phase, v=bw),
                    AF.Exp, bias=biasK[:])

            def unit_gold_emit(c, half, stage):
                # emission-gold dot: small DVE mult slices + one reduce
                t0 = c * chunk
                w = (min(s_steps, t0 + chunk) - t0) * BL
                ohem = ohemA if half == 0 else ohemB
                raw = raws[(c, half)]
                key = (c, half)
                if stage in (0, 1):
                    hw_ = w // 2
                    lo = stage * hw_
                    if stage == 0:
                        scr = scrp.tile([128, cw], F32, name=f"sc{key}", tag="scr")
                        raws[("scr", c, half)] = scr
                    scr = raws[("scr", c, half)]
                    eng = nc.gpsimd if pool_mult else nc.vector
                    eng.tensor_tensor(
                        scr[:, lo:lo + hw_], raw[:, lo:lo + hw_],
                        ohem[:, t0 * BL + lo:t0 * BL + lo + hw_], OP.mult)
                else:
                    # stage 2..5: quarter reduces (keep scan-DVE gaps small)
                    scr = raws[("scr", c, half)]
                    q = stage - 2
                    qw = w // 4
                    col = 8 * c + 4 * half + q
                    nc.vector.reduce_sum(ep[:, col:col + 1],
                                         scr[:, q * qw:(q + 1) * qw],
                                         axis=mybir.AxisListType.X)

            # ---------------- gold pair matmuls ----------------
            Gb = ps_g.tile([128, 2 * T], F32, tag="Gb")
            G0 = Gb[:, 0:T]
            G1 = Gb[:, T:2 * T]

            def unit_gold_pair(j):
                pv = ohpv_s[:, j * T:(j + 1) * T]
                cu = ohc_s[:, j * T:(j + 1) * T]
                nc.tensor.matmul(G0, lhsT=pv[:, 0:128], rhs=cu,
                                 start=(j == 0), stop=(j == nj - 1),
                                 skip_group_check=True)
                nc.tensor.matmul(G1, lhsT=pv[:, 128:256], rhs=cu,
                                 start=(j == 0), stop=(j == nj - 1),
                                 skip_group_check=True)

            ro = {}

            def unit_readout_prep(stage):
                if stage == 0:
                    Lb = sp.tile([BL, 1], F32, tag="Lb")
                    nc.vector.reduce_sum(Lb[:], maskf[:], axis=mybir.AxisListType.X)
                    ro["Lb"] = Lb
                else:
                    endrows = sp.tile([BL, T], F32, tag="endrows")
                    nc.gpsimd.indirect_dma_start(
                        out=endrows[:], out_offset=None, in_=trans[:],
                        in_offset=bass.IndirectOffsetOnAxis(ap=ro["endid"][:, 0:1], axis=0))
                    ro["endrows"] = endrows

            def unit_gold_post(stage):
                if stage == 0:
                    gscr = sp.tile([128, T], F32, tag="gscr")
                    tg0 = sp.tile([128, 1], F32, tag="tg0")
                    nc.vector.tensor_tensor(gscr[:], G0, trA[:], OP.mult)
                    nc.vector.reduce_sum(tg0[:], gscr[:], axis=mybir.AxisListType.X)
                    ro["tg0"] = tg0
                elif stage == 1:
                    gscr2 = sp.tile([128, T], F32, tag="gscr2")
                    tg1 = sp.tile([128, 1], F32, tag="tg1")
                    nc.vector.tensor_tensor(gscr2[:], G1, trB[:], OP.mult)
                    nc.vector.reduce_sum(tg1[:], gscr2[:], axis=mybir.AxisListType.X)
                    ro["tg1"] = tg1
                else:
                    ones128 = sp.tile([128, 1], F32, tag="ones128")
                    nc.gpsimd.memset(ones128[:], 1.0)
                    r128 = sp.tile([128, 3], F32, tag="r128")
                    nc.vector.reduce_sum(r128[:, 0:1], ep[:], axis=mybir.AxisListType.X)
                    nc.vector.tensor_copy(r128[:, 1:2], ro["tg0"][:])
                    nc.vector.tensor_copy(r128[:, 2:3], ro["tg1"][:])
                    p128 = ps_fin.tile([1, 3], F32, tag="p128")
                    nc.tensor.matmul(p128[:], lhsT=ones128[:, 0:1], rhs=r128[:],
                                     start=True, stop=True)
                    ro["p128"] = p128

            def unit_oh_dma(which):
                if which == 0:
                    nc.sync.dma_start(out=ohemA[:], in_=ohemA_d[:])
                elif which == 1:
                    nc.sync.dma_start(out=ohemB[:], in_=ohemB_d[:])
                elif which == 2:
                    nc.sync.dma_start(out=ohpv_s[:], in_=ohprev[:])
                else:
                    nc.sync.dma_start(out=ohc_s[:], in_=ohcur[:])

            # ---------------- schedule ----------------
            for half in (0, 1):
                unit_exp_mini(half)

            sched = {t: [] for t in range(1, s_steps + 1)}

            def at(t, fn, *a):
                sched[min(s_steps, max(1, t))].append((fn, a))

            ro["endid"] = endid_t
            ro["hoff"] = hoff_t
            at(2, unit_exp, 0, 0)
            at(3, unit_exp, 0, 1)
            for i in range(4):
                at(4 + i, unit_oh_dma, i)
            for st in range(2):
                at(100 + 6 * st, unit_readout_prep, st)
            for c in range(1, nch):
                base = (c - 1) * chunk
                for half in (0, 1):
                    if c + 1 < nch:
                        at(base + 8 + half, unit_dma, c + 1, half)
                    at(base + 16 + 8 * half, unit_exp, c, half)
            # emission-gold DVE slices for chunk c run while chunk c+1 scans
            for c in range(nch):
                base = c * chunk + chunk // 2
                for half in (0, 1):
                    at(base + 4 + 24 * half, unit_gold_emit, c, half, 0)
                    at(base + 8 + 24 * half, unit_gold_emit, c, half, 1)
                    for q in range(4):
                        at(base + 12 + 24 * half + 3 * q, unit_gold_emit, c, half, 2 + q)
            if gold_inline:
                lead = s_steps // 4
                spacing = max(1, (s_steps - lead - 24) // nj)
                for j in range(nj):
                    at(lead + j * spacing, unit_gold_pair, j)

            # ---------------- the scan ----------------
            H = pp.tile([128, nsl], F32, tag="H")

            ws = []
            for g in range(phase):
                w0 = wp.tile([128, gw2], BF16, name=f"w{g}", tag=f"w{g}")
                nc.vector.tensor_scalar(w0[:, 0:bw], expABv[:, 0, g, 0, :],
                                        estA[:, 0:1], None, OP.mult)
                nc.vector.tensor_scalar(w0[:, bw:gw2], expABv[:, 0, g, 1, :],
                                        estB[:, 0:1], None, OP.mult)
                ws.append(w0)

            strips = [None] * phase
            for t in range(1, s_steps + 1):
                for fn, a in sched[t]:
                    fn(*a)
                j = (t - 1) % kcap
                for g in range(phase):
                    if j == 0:
                        strips[g] = ps_strip.tile([128, kcap * gw2], F32,
                                                  name=f"st{g}", tag=f"st{g}")
                    strip = strips[g]
                    sl = strip[:, j * gw2:(j + 1) * gw2]
                    wg = ws[g]
                    nc.tensor.matmul(sl[:, 0:bw], lhsT=EA[:, 0:128], rhs=wg[:, 0:bw],
                                     start=True, stop=False, skip_group_check=True)
                    nc.tensor.matmul(sl[:, 0:bw], lhsT=EB[:, 0:128], rhs=wg[:, bw:gw2],
                                     start=False, stop=True, skip_group_check=True)
                    nc.tensor.matmul(sl[:, bw:gw2], lhsT=EA[:, 128:256], rhs=wg[:, 0:bw],
                                     start=True, stop=False, skip_group_check=True)
                    nc.tensor.matmul(sl[:, bw:gw2], lhsT=EB[:, 128:256], rhs=wg[:, bw:gw2],
                                     start=False, stop=True, skip_group_check=True)
                    if t < s_steps:
                        wg2t = wp.tile([128, gw2], BF16, name=f"w{g}", tag=f"w{g}")
                        off = (t * phase + g) * gw2
                        nc.vector.tensor_tensor(
                            wg2t[:], sl[:], expAB[:, off:off + gw2], OP.mult)
                        ws[g] = wg2t
                    if j == kcap - 1 or t == s_steps:
                        tlo = t - 1 - j
                        stv = strip[96:128, 0:(j + 1) * gw2].rearrange(
                            "p (k h w) -> p k h w", h=2, w=bw)
                        hv = H[96:128, tlo * BL:t * BL].rearrange(
                            "p (k b) -> p k b", b=BL)
                        nc.scalar.copy(hv[:, :, g * bw:(g + 1) * bw],
                                       stv[:, :, 1, :])
                        if g == phase - 1:
                            nc.sync.dma_start(
                                out=h_d[tlo * BL:t * BL, 0],
                                in_=H[127:128, tlo * BL:t * BL])

            # ---------------- post-loop ----------------
            # gold-matrix contraction AFTER the scan: scheduling it in-scan
            # raced the last emission-gold units' ep writes (Tile missed the
            # overlapping-slice dependency) and made results nondeterministic
            for st in range(3):
                unit_gold_post(st)
            hstop = sp.tile([BL, 1], F32, tag="hstop")
            nc.gpsimd.indirect_dma_start(
                out=hstop[:], out_offset=None, in_=h_d[:],
                in_offset=bass.IndirectOffsetOnAxis(ap=ro["hoff"][:, 0:1], axis=0))

            lnh = sp.tile([BL, 1], F32, tag="lnh")
            nc.scalar.activation(lnh[:], hstop[:], AF.Ln)
            fwdb = sp.tile([BL, 1], F32, tag="fwdb")
            kl = sp.tile([BL, 1], F32, tag="kl")
            nc.vector.tensor_scalar(kl[:], ro["Lb"][:], KAPPA, None, OP.mult)
            nc.vector.tensor_tensor(fwdb[:], kl[:], lnh[:], OP.add)

            # gold matrix already contracted in-scan (unit_gold_post)
            p128 = ro["p128"]

            r16 = sp.tile([BL, 2], F32, tag="r16")
            nc.vector.tensor_copy(r16[:, 0:1], fwdb[:])
            nc.vector.tensor_copy(r16[:, 1:2], ro["endrows"][:, STOP:STOP + 1])
            ones16 = sp.tile([BL, 1], F32, tag="ones16")
            nc.gpsimd.memset(ones16[:], 1.0)
            p16 = ps_fin.tile([1, 2], F32, tag="p16")
            nc.tensor.matmul(p16[:], lhsT=ones16[:, 0:1], rhs=r16[:], start=True, stop=True)

            s128 = sp.tile([1, 3], F32, tag="s128")
            s16 = sp.tile([1, 2], F32, tag="s16")
            nc.vector.tensor_copy(s128[:], p128[:])
            nc.vector.tensor_copy(s16[:], p16[:])
            gold128 = sp.tile([1, 1], F32, tag="gold128")
            nc.vector.reduce_sum(gold128[:], s128[:], axis=mybir.AxisListType.X)
            fin0 = sp.tile([1, 1], F32, tag="fin0")
            nc.vector.tensor_tensor(fin0[:], s16[:, 0:1], s16[:, 1:2], OP.subtract)
            fin = sp.tile([1, 1], F32, tag="fin")
            nc.vector.tensor_tensor(fin[:], fin0[:], gold128[:], OP.subtract)
            nc.sync.dma_start(out=out[:], in_=fin[:])

    if do_compile:
        nc.compile()
    return nc


def make_in_maps(feats, transitions, tags, mask, s_steps=S):
    feats = np.asarray(feats, dtype=np.float32)
    transitions = np.asarray(transitions, dtype=np.float32)
    tags = np.asarray(tags).astype(np.int32)
    mask = np.asarray(mask).astype(np.int32)
    nj = (BL * s_steps) // 128
    nsl = s_steps * BL
    eye = np.eye(T, dtype=F8NP)
    zrow = np.zeros((1, T), dtype=F8NP)
    ohtab = np.concatenate([eye, zrow], axis=0)
    in_maps = []
    for c in range(NCORES):
        bs = slice(c * BL, (c + 1) * BL)
        f = np.ascontiguousarray(feats[bs, :s_steps, :].transpose(2, 1, 0))
        tg = np.ascontiguousarray(tags[bs, :s_steps])
        mk = np.ascontiguousarray(mask[bs, :s_steps])
        pv = np.concatenate(
            [np.full((BL, 1), START, np.int32), tg[:, :-1]], axis=1)
        tgm = np.where(mk > 0, tg, T)
        pvm = np.where(mk > 0, pv, T)

        def btT(x):
            return x.reshape(-1).reshape(-1, 128).T

        ohc = np.ascontiguousarray(ohtab[btT(tgm)].reshape(128, nj * T))
        ohp = np.ascontiguousarray(ohtab[btT(pvm)].reshape(128, nj * T))
        # emission one-hots: [state(128) x (t,b)] per state-half
        tb = np.ascontiguousarray(tgm.T).reshape(nsl)      # (t, b) order
        ohem = np.ascontiguousarray(ohtab[tb][:, :].T)     # [T, nsl]
        # index metadata: end-tag ids and stop-row gather offsets
        L = mk.sum(axis=1)
        endid = np.take_along_axis(tg, (L - 1)[:, None], axis=1).astype(np.int32)
        hoff = ((L - 1) * BL + np.arange(BL)).astype(np.int32)[:, None]
        in_maps.append({
            "feats_sm": f,
            "trans": transitions,
            "tags_bm": tg,
            "mask_bm": mk,
            "ohprev": ohp,
            "ohcur": ohc,
            "ohemA": np.ascontiguousarray(ohem[0:128]),
            "ohemB": np.ascontiguousarray(ohem[128:256]),
            "endid": endid,
            "hoff": hoff,
        })
    return in_maps


_CACHE = {}


def kernel(**inputs):
    if "nc" not in _CACHE:
        _CACHE["nc"] = build_program()
    nc = _CACHE["nc"]
    in_maps = make_in_maps(inputs["feats"], inputs["transitions"],
                           inputs["tags"], inputs["mask"])
    res = run_bass_kernel_spmd(nc, in_maps, core_ids=list(range(NCORES)))
    total = np.float64(0.0)
    for r in res.results:
        total += np.float64(r["out"].reshape(()))
    return np.asarray(total, dtype=np.float32).reshape(())


# revision 13
# speedup vs baseline: 1.1781x; 1.1781x over previous
"""CRF NLL loss on 8 NeuronCores — v3: scan-centric schedule.

Same math as v2 (fp8 stationary weights E'=exp(trans), kappa folded into
emissions, batched strip capture). v3:
 - feats streaming (exp), emission-gold dot and gold pair matmuls are
   INTERLEAVED into the scan loop to fill engine idle time; the scan chain
   owns only the per-step DVE pointwise.
 - ALL one-hots are host-encoded fp8 (GPSIMD is_equal is ~15.6us per
   [128,1024] op on the Q7 software engine — it backpressured the whole
   pipeline in v2). Emission-gold dot = small DVE mult+reduce slices.
 - flat 2D access patterns on the scan-critical DVE op.
 - masked readout via a 16-element indirect gather from the stop-row
   history instead of a transposed (fragmented) 32KB DMA + [16,512] Ln.
"""

import numpy as np

import concourse.bass as bass
import concourse.tile as tile
from concourse import bacc, mybir
from concourse.bass_utils import run_bass_kernel_spmd

F32 = mybir.dt.float32
BF16 = mybir.dt.bfloat16
F8 = mybir.dt.float8e4
I32 = mybir.dt.int32
I16 = mybir.dt.int16
AF = mybir.ActivationFunctionType
OP = mybir.AluOpType

B, S, T = 128, 512, 256
NCORES = 8
BL = B // NCORES
START, STOP = T - 2, T - 1
KAPPA = float(np.log(T) + 0.5)
NJ = (BL * S) // 128

F8NP = mybir.dt.np(F8)


def build_program(s_steps=S, chunk=64, kcap=None, f8=True, gold_inline=True,
                  phase=1, strip_bufs=None, pool_mult=True, do_compile=True):
    if kcap is None:
        kcap = 32
    if strip_bufs is None:
        strip_bufs = 2 if phase <= 2 else 1
    nc = bacc.Bacc("TRN2", target_bir_lowering=False, debug=False,
                   num_devices=NCORES)

    feats_sm = nc.dram_tensor("feats_sm", [T, s_steps, BL], F32, kind="ExternalInput")
    trans = nc.dram_tensor("trans", [T, T], F32, kind="ExternalInput")
    tags_bm = nc.dram_tensor("tags_bm", [BL, s_steps], I32, kind="ExternalInput")
    mask_bm = nc.dram_tensor("mask_bm", [BL, s_steps], I32, kind="ExternalInput")
    nj = (BL * s_steps) // 128
    ohprev = nc.dram_tensor("ohprev", [128, nj * T], F8, kind="ExternalInput")
    ohcur = nc.dram_tensor("ohcur", [128, nj * T], F8, kind="ExternalInput")
    nsl = s_steps * BL
    ohemA_d = nc.dram_tensor("ohemA", [128, nsl], F8, kind="ExternalInput")
    ohemB_d = nc.dram_tensor("ohemB", [128, nsl], F8, kind="ExternalInput")
    # host-precomputed index metadata (like prevT in v1): end-tag ids and
    # stop-row gather offsets (len_b-1)*BL + b
    endid_d = nc.dram_tensor("endid", [BL, 1], I32, kind="ExternalInput")
    hoff_d = nc.dram_tensor("hoff", [BL, 1], I32, kind="ExternalInput")
    out = nc.dram_tensor("out", [1, 1], F32, kind="ExternalOutput")

    h_d = nc.dram_tensor("h_d", [s_steps * BL, 1], F32)   # stop-row history (t,b)

    nch = (s_steps + chunk - 1) // chunk
    cw = chunk * BL                     # free width of one half-chunk

    with tile.TileContext(nc) as tc:
        with (
            tc.tile_pool(name="persist", bufs=1) as pp,
            tc.tile_pool(name="raw", bufs=6) as rawp,
            tc.tile_pool(name="scr", bufs=2) as scrp,
            tc.tile_pool(name="w", bufs=2) as wp,
            tc.tile_pool(name="small", bufs=1) as sp,
            tc.tile_pool(name="ps_strip", bufs=strip_bufs, space="PSUM") as ps_strip,
            tc.tile_pool(name="ps_g", bufs=1, space="PSUM") as ps_g,
            tc.tile_pool(name="ps_fin", bufs=1, space="PSUM") as ps_fin,
        ):
            # ---------------- feats chunk 0/1 DMA first (gates the scan) ----
            raws = {}

            def unit_dma(c, half):
                t0 = c * chunk
                t1c = min(s_steps, t0 + chunk)
                w = (t1c - t0) * BL
                raw = rawp.tile([128, cw], F32, tag="raw")
                nc.sync.dma_start(
                    out=raw[:, 0:w],
                    in_=feats_sm[half * 128:(half + 1) * 128, t0:t1c, :]
                    .rearrange("p a b -> p (a b)"))
                raws[(c, half)] = raw

            # mini-DMA for steps 0..7 only: the scan can start as soon as
            # these ~64KB land, instead of waiting for the full first chunk
            minis = []
            for half in (0, 1):
                rm = rawp.tile([128, 8 * BL], F32, tag="rawm")
                nc.sync.dma_start(
                    out=rm[:],
                    in_=feats_sm[half * 128:(half + 1) * 128, 0:8, :]
                    .rearrange("p a b -> p (a b)"))
                minis.append(rm)
            for half in (0, 1):
                unit_dma(0, half)

            # ---------------- constants / transition prep ----------------
            trA = pp.tile([128, T], F32, tag="trA")
            trB = pp.tile([128, T], F32, tag="trB")
            nc.sync.dma_start(out=trA[:], in_=trans[0:128, :])
            nc.sync.dma_start(out=trB[:], in_=trans[128:256, :])
            WD = F8 if f8 else BF16
            EA = pp.tile([128, T], WD, tag="EA")
            EB = pp.tile([128, T], WD, tag="EB")
            biasK = sp.tile([128, 1], F32, tag="biasK")
            nc.gpsimd.memset(biasK[:], -KAPPA)
            if f8:
                nc.scalar.activation(EA[:], trA[:], AF.Exp)
                nc.scalar.activation(EB[:], trB[:], AF.Exp)
            else:
                nc.scalar.activation(EA[:], trA[:], AF.Exp, bias=biasK[:])
                nc.scalar.activation(EB[:], trB[:], AF.Exp, bias=biasK[:])

            stA = sp.tile([128, 1], F32, tag="stA")
            stB = sp.tile([128, 1], F32, tag="stB")
            nc.sync.dma_start(out=stA[:], in_=trans[START:START + 1, 0:128])
            nc.sync.dma_start(out=stB[:], in_=trans[START:START + 1, 128:256])
            estA = sp.tile([128, 1], F32, tag="estA")
            estB = sp.tile([128, 1], F32, tag="estB")
            nc.scalar.activation(estA[:], stA[:], AF.Exp)
            nc.scalar.activation(estB[:], stB[:], AF.Exp)

            # ---------------- tags / mask prep ----------------
            mk = sp.tile([BL, s_steps], I32, tag="mk")
            nc.sync.dma_start(out=mk[:], in_=mask_bm[:])
            maskf = sp.tile([BL, s_steps], F32, tag="maskf")
            nc.vector.tensor_copy(maskf[:], mk[:])
            endid_t = sp.tile([BL, 1], I32, tag="endid_t")
            nc.sync.dma_start(out=endid_t[:], in_=endid_d[:])
            hoff_t = sp.tile([BL, 1], I32, tag="hoff_t")
            nc.sync.dma_start(out=hoff_t[:], in_=hoff_d[:])

            unit_dma(1, 0)
            unit_dma(1, 1)

            # one-hot tiles (host-encoded fp8)
            ohpv_s = pp.tile([128, nj * T], F8, tag="ohpv_s")
            ohc_s = pp.tile([128, nj * T], F8, tag="ohc_s")
            ohemA = pp.tile([128, nsl], F8, tag="ohemA")
            ohemB = pp.tile([128, nsl], F8, tag="ohemB")

            # ---------------- streaming units (emitted interleaved) --------
            # emissions: (p, t, g, h, w): chain g's step slice is flat [128, 2*bw]
            bw = BL // phase
            gw2 = 2 * bw
            expAB = pp.tile([128, 2 * nsl], BF16, tag="expAB")
            expABv = expAB[:].rearrange("p (t g h w) -> p t g h w",
                                        g=phase, h=2, w=bw)
            ep = sp.tile([128, 8 * nch], F32, tag="ep")   # emission-gold partials

            def unit_exp(c, half):
                t0 = c * chunk
                t1c = min(s_steps, t0 + chunk)
                raw = raws[(c, half)]
                lo = 0
                if c == 0:
                    lo = 8 * BL          # steps 0..7 come from the mini tiles
                    t0 = 8
                w = t1c * BL - c * chunk * BL - lo
                nc.scalar.activation(
                    expABv[:, t0:t1c, :, half, :],
                    raw[:, lo:lo + w].rearrange("p (a g v) -> p a g v",
                                                g=phase, v=bw),
                    AF.Exp, bias=biasK[:])

            def unit_exp_mini(half):
                nc.scalar.activation(
                    expABv[:, 0:8, :, half, :],
                    minis[half][:].rearrange("p (a g v) -> p a g v",
                                             g=phase, v=bw),
                    AF.Exp, bias=biasK[:])

            def unit_gold_emit(c, half, stage):
                # emission-gold dot: small DVE mult slices + one reduce
                t0 = c * chunk
                w = (min(s_steps, t0 + chunk) - t0) * BL
                ohem = ohemA if half == 0 else ohemB
                raw = raws[(c, half)]
                key = (c, half)
                if stage in (0, 1):
                    hw_ = w // 2
                    lo = stage * hw_
                    if stage == 0:
                        scr = scrp.tile([128, cw], F32, name=f"sc{key}", tag="scr")
                        raws[("scr", c, half)] = scr
                    scr = raws[("scr", c, half)]
                    eng = nc.gpsimd if pool_mult else nc.vector
                    eng.tensor_tensor(
                        scr[:, lo:lo + hw_], raw[:, lo:lo + hw_],
                        ohem[:, t0 * BL + lo:t0 * BL + lo + hw_], OP.mult)
                else:
                    # stage 2..5: quarter reduces (keep scan-DVE gaps small)
                    scr = raws[("scr", c, half)]
                    q = stage - 2
                    qw = w // 4
                    col = 8 * c + 4 * half + q
                    nc.vector.reduce_sum(ep[:, col:col + 1],
                                         scr[:, q * qw:(q + 1) * qw],
                                         axis=mybir.AxisListType.X)

            # ---------------- gold pair matmuls ----------------
            Gb = ps_g.tile([128, 2 * T], F32, tag="Gb")
            G0 = Gb[:, 0:T]
            G1 = Gb[:, T:2 * T]

            def unit_gold_pair(j):
                pv = ohpv_s[:, j * T:(j + 1) * T]
                cu = ohc_s[:, j * T:(j + 1) * T]
                nc.tensor.matmul(G0, lhsT=pv[:, 0:128], rhs=cu,
                                 start=(j == 0), stop=(j == nj - 1),
                                 skip_group_check=True)
                nc.tensor.matmul(G1, lhsT=pv[:, 128:256], rhs=cu,
                                 start=(j == 0), stop=(j == nj - 1),
                                 skip_group_check=True)

            ro = {}

            def unit_readout_prep(stage):
                if stage == 0:
                    Lb = sp.tile([BL, 1], F32, tag="Lb")
                    nc.vector.reduce_sum(Lb[:], maskf[:], axis=mybir.AxisListType.X)
                    ro["Lb"] = Lb
                else:
                    endrows = sp.tile([BL, T], F32, tag="endrows")
                    nc.gpsimd.indirect_dma_start(
                        out=endrows[:], out_offset=None, in_=trans[:],
                        in_offset=bass.IndirectOffsetOnAxis(ap=ro["endid"][:, 0:1], axis=0))
                    ro["endrows"] = endrows

            def unit_gold_post(stage):
                if stage == 0:
                    gscr = sp.tile([128, T], F32, tag="gscr")
                    tg0 = sp.tile([128, 1], F32, tag="tg0")
                    nc.vector.tensor_tensor(gscr[:], G0, trA[:], OP.mult)
                    nc.vector.reduce_sum(tg0[:], gscr[:], axis=mybir.AxisListType.X)
                    ro["tg0"] = tg0
                elif stage == 1:
                    gscr2 = sp.tile([128, T], F32, tag="gscr2")
                    tg1 = sp.tile([128, 1], F32, tag="tg1")
                    nc.vector.tensor_tensor(gscr2[:], G1, trB[:], OP.mult)
                    nc.vector.reduce_sum(tg1[:], gscr2[:], axis=mybir.AxisListType.X)
                    ro["tg1"] = tg1
                else:
                    ones128 = sp.tile([128, 1], F32, tag="ones128")
                    nc.gpsimd.memset(ones128[:], 1.0)
                    r128 = sp.tile([128, 3], F32, tag="r128")
                    nc.vector.reduce_sum(r128[:, 0:1], ep[:], axis=mybir.AxisListType.X)
                    nc.vector.tensor_copy(r128[:, 1:2], ro["tg0"][:])
                    nc.vector.tensor_copy(r128[:, 2:3], ro["tg1"][:])
                    p128 = ps_fin.tile([1, 3], F32, tag="p128")
                    nc.tensor.matmul(p128[:], lhsT=ones128[:, 0:1], rhs=r128[:],
                                     start=True, stop=True)
                    ro["p128"] = p128

            def unit_oh_dma(which):
                if which == 0:
                    nc.sync.dma_start(out=ohemA[:], in_=ohemA_d[:])
                elif which == 1:
                    nc.sync.dma_start(out=ohemB[:], in_=ohemB_d[:])
                elif which == 2:
                    nc.sync.dma_start(out=ohpv_s[:], in_=ohprev[:])
                else:
                    nc.sync.dma_start(out=ohc_s[:], in_=ohcur[:])

            # ---------------- schedule ----------------
            for half in (0, 1):
                unit_exp_mini(half)

            sched = {t: [] for t in range(1, s_steps + 1)}

            def at(t, fn, *a):
                sched[min(s_steps, max(1, t))].append((fn, a))

            ro["endid"] = endid_t
            ro["hoff"] = hoff_t
            at(2, unit_exp, 0, 0)
            at(3, unit_exp, 0, 1)
            for i in range(4):
                at(4 + i, unit_oh_dma, i)
            for st in range(2):
                at(100 + 6 * st, unit_readout_prep, st)
            for c in range(1, nch):
                base = (c - 1) * chunk
                for half in (0, 1):
                    if c + 1 < nch:
                        at(base + 8 + half, unit_dma, c + 1, half)
                    at(base + 16 + 8 * half, unit_exp, c, half)
            # emission-gold DVE slices for chunk c run while chunk c+1 scans
            for c in range(nch):
                base = c * chunk + chunk // 2
                for half in (0, 1):
                    at(base + 4 + 24 * half, unit_gold_emit, c, half, 0)
                    at(base + 8 + 24 * half, unit_gold_emit, c, half, 1)
                    for q in range(4):
                        at(base + 12 + 24 * half + 3 * q, unit_gold_emit, c, half, 2 + q)
            if gold_inline:
                lead = s_steps // 4
                spacing = max(1, (s_steps - lead - 24) // nj)
                for j in range(nj):
                    at(lead + j * spacing, unit_gold_pair, j)

            # ---------------- the scan ----------------
            H = pp.tile([128, nsl], F32, tag="H")

            ws = []
            for g in range(phase):
                w0 = wp.tile([128, gw2], BF16, name=f"w{g}", tag=f"w{g}")
                nc.vector.tensor_scalar(w0[:, 0:bw], expABv[:, 0, g, 0, :],
                                        estA[:, 0:1], None, OP.mult)
                nc.vector.tensor_scalar(w0[:, bw:gw2], expABv[:, 0, g, 1, :],
                                        estB[:, 0:1], None, OP.mult)
                ws.append(w0)

            strips = [None] * phase
            for t in range(1, s_steps + 1):
                for fn, a in sched[t]:
                    fn(*a)
                j = (t - 1) % kcap
                for g in range(phase):
                    if j == 0:
                        strips[g] = ps_strip.tile([128, kcap * gw2], F32,
                                                  name=f"st{g}", tag=f"st{g}")
                    strip = strips[g]
                    sl = strip[:, j * gw2:(j + 1) * gw2]
                    wg = ws[g]
                    nc.tensor.matmul(sl[:, 0:bw], lhsT=EA[:, 0:128], rhs=wg[:, 0:bw],
                                     start=True, stop=False, skip_group_check=True)
                    nc.tensor.matmul(sl[:, 0:bw], lhsT=EB[:, 0:128], rhs=wg[:, bw:gw2],
                                     start=False, stop=True, skip_group_check=True)
                    nc.tensor.matmul(sl[:, bw:gw2], lhsT=EA[:, 128:256], rhs=wg[:, 0:bw],
                                     start=True, stop=False, skip_group_check=True)
                    nc.tensor.matmul(sl[:, bw:gw2], lhsT=EB[:, 128:256], rhs=wg[:, bw:gw2],
                                     start=False, stop=True, skip_group_check=True)
                    if t < s_steps:
                        wg2t = wp.tile([128, gw2], BF16, name=f"w{g}", tag=f"w{g}")
                        off = (t * phase + g) * gw2
                        nc.vector.tensor_tensor(
                            wg2t[:], sl[:], expAB[:, off:off + gw2], OP.mult)
                        ws[g] = wg2t
                    if j == kcap - 1 or t == s_steps:
                        tlo = t - 1 - j
                        stv = strip[96:128, 0:(j + 1) * gw2].rearrange(
                            "p (k h w) -> p k h w", h=2, w=bw)
                        hv = H[96:128, tlo * BL:t * BL].rearrange(
                            "p (k b) -> p k b", b=BL)
                        nc.scalar.copy(hv[:, :, g * bw:(g + 1) * bw],
                                       stv[:, :, 1, :])
                        if g == phase - 1:
                            nc.sync.dma_start(
                                out=h_d[tlo * BL:t * BL, 0],
                                in_=H[127:128, tlo * BL:t * BL])

            # ---------------- post-loop ----------------
            # gold-matrix contraction AFTER the scan: scheduling it in-scan
            # raced the last emission-gold units' ep writes (Tile missed the
            # overlapping-slice dependency) and made results nondeterministic
            for st in range(3):
                unit_gold_post(st)
            hstop = sp.tile([BL, 1], F32, tag="hstop")
            nc.gpsimd.indirect_dma_start(
                out=hstop[:], out_offset=None, in_=h_d[:],
                in_offset=bass.IndirectOffsetOnAxis(ap=ro["hoff"][:, 0:1], axis=0))

            lnh = sp.tile([BL, 1], F32, tag="lnh")
            nc.scalar.activation(lnh[:], hstop[:], AF.Ln)
            fwdb = sp.tile([BL, 1], F32, tag="fwdb")
            kl = sp.tile([BL, 1], F32, tag="kl")
            nc.vector.tensor_scalar(kl[:], ro["Lb"][:], KAPPA, None, OP.mult)
            nc.vector.tensor_tensor(fwdb[:], kl[:], lnh[:], OP.add)

            # gold matrix already contracted in-scan (unit_gold_post)
            p128 = ro["p128"]

            r16 = sp.tile([BL, 2], F32, tag="r16")
            nc.vector.tensor_copy(r16[:, 0:1], fwdb[:])
            nc.vector.tensor_copy(r16[:, 1:2], ro["endrows"][:, STOP:STOP + 1])
            ones16 = sp.tile([BL, 1], F32, tag="ones16")
            nc.gpsimd.memset(ones16[:], 1.0)
            p16 = ps_fin.tile([1, 2], F32, tag="p16")
            nc.tensor.matmul(p16[:], lhsT=ones16[:, 0:1], rhs=r16[:], start=True, stop=True)

            s128 = sp.tile([1, 3], F32, tag="s128")
            s16 = sp.tile([1, 2], F32, tag="s16")
            nc.vector.tensor_copy(s128[:], p128[:])
            nc.vector.tensor_copy(s16[:], p16[:])
            gold128 = sp.tile([1, 1], F32, tag="gold128")
            nc.vector.reduce_sum(gold128[:], s128[:], axis=mybir.AxisListType.X)
            fin0 = sp.tile([1, 1], F32, tag="fin0")
            nc.vector.tensor_tensor(fin0[:], s16[:, 0:1], s16[:, 1:2], OP.subtract)
            fin = sp.tile([1, 1], F32, tag="fin")
            nc.vector.tensor_tensor(fin[:], fin0[:], gold128[:], OP.subtract)
            nc.sync.dma_start(out=out[:], in_=fin[:])

    if do_compile:
        nc.compile()
    return nc


def make_in_maps(feats, transitions, tags, mask, s_steps=S):
    feats = np.asarray(feats, dtype=np.float32)
    transitions = np.asarray(transitions, dtype=np.float32)
    tags = np.asarray(tags).astype(np.int32)
    mask = np.asarray(mask).astype(np.int32)
    nj = (BL * s_steps) // 128
    nsl = s_steps * BL
    eye = np.eye(T, dtype=F8NP)
    zrow = np.zeros((1, T), dtype=F8NP)
    ohtab = np.concatenate([eye, zrow], axis=0)
    in_maps = []
    for c in range(NCORES):
        bs = slice(c * BL, (c + 1) * BL)
        f = np.ascontiguousarray(feats[bs, :s_steps, :].transpose(2, 1, 0))
        tg = np.ascontiguousarray(tags[bs, :s_steps])
        mk = np.ascontiguousarray(mask[bs, :s_steps])
        pv = np.concatenate(
            [np.full((BL, 1), START, np.int32), tg[:, :-1]], axis=1)
        tgm = np.where(mk > 0, tg, T)
        pvm = np.where(mk > 0, pv, T)

        def btT(x):
            return x.reshape(-1).reshape(-1, 128).T

        ohc = np.ascontiguousarray(ohtab[btT(tgm)].reshape(128, nj * T))
        ohp = np.ascontiguousarray(ohtab[btT(pvm)].reshape(128, nj * T))
        # emission one-hots: [state(128) x (t,b)] per state-half
        tb = np.ascontiguousarray(tgm.T).reshape(nsl)      # (t, b) order
        ohem = np.ascontiguousarray(ohtab[tb][:, :].T)     # [T, nsl]
        # index metadata: end-tag ids and stop-row gather offsets
        L = mk.sum(axis=1)
        endid = np.take_along_axis(tg, (L - 1)[:, None], axis=1).astype(np.int32)
        hoff = ((L - 1) * BL + np.arange(BL)).astype(np.int32)[:, None]
        in_maps.append({
            "feats_sm": f,
            "trans": transitions,
            "tags_bm": tg,
            "mask_bm": mk,
            "ohprev": ohp,
            "ohcur": ohc,
            "ohemA": np.ascontiguousarray(ohem[0:128]),
            "ohemB": np.ascontiguousarray(ohem[128:256]),
            "endid": endid,
            "hoff": hoff,
        })
    return in_maps


_CACHE = {}


def kernel(**inputs):
    if "nc" not in _CACHE:
        _CACHE["nc"] = build_program()
    nc = _CACHE["nc"]
    in_maps = make_in_maps(inputs["feats"], inputs["transitions"],
                           inputs["tags"], inputs["mask"])
    res = run_bass_kernel_spmd(nc, in_maps, core_ids=list(range(NCORES)))
    total = np.float64(0.0)
    for r in res.results:
        total += np.float64(r["out"].reshape(()))
    return np.asarray(total, dtype=np.float32).reshape(())


# revision 15
# speedup vs baseline: 1.1841x; 1.0051x over previous
"""CRF NLL loss on 8 NeuronCores — v3: scan-centric schedule.

Same math as v2 (fp8 stationary weights E'=exp(trans), kappa folded into
emissions, batched strip capture). v3:
 - feats streaming (exp), emission-gold dot and gold pair matmuls are
   INTERLEAVED into the scan loop to fill engine idle time; the scan chain
   owns only the per-step DVE pointwise.
 - ALL one-hots are host-encoded fp8 (GPSIMD is_equal is ~15.6us per
   [128,1024] op on the Q7 software engine — it backpressured the whole
   pipeline in v2). Emission-gold dot = small DVE mult+reduce slices.
 - flat 2D access patterns on the scan-critical DVE op.
 - masked readout via a 16-element indirect gather from the stop-row
   history instead of a transposed (fragmented) 32KB DMA + [16,512] Ln.
"""

import numpy as np

import concourse.bass as bass
import concourse.tile as tile
from concourse import bacc, mybir
from concourse.bass_utils import run_bass_kernel_spmd

F32 = mybir.dt.float32
BF16 = mybir.dt.bfloat16
F8 = mybir.dt.float8e4
I32 = mybir.dt.int32
I16 = mybir.dt.int16
AF = mybir.ActivationFunctionType
OP = mybir.AluOpType

B, S, T = 128, 512, 256
NCORES = 8
BL = B // NCORES
START, STOP = T - 2, T - 1
KAPPA = float(np.log(T) + 0.5)
NJ = (BL * S) // 128

F8NP = mybir.dt.np(F8)


def build_program(s_steps=S, chunk=64, kcap=None, f8=True, gold_inline=True,
                  phase=1, strip_bufs=None, pool_mult=True, do_compile=True):
    if kcap is None:
        kcap = 32
    if strip_bufs is None:
        strip_bufs = 2 if phase <= 2 else 1
    nc = bacc.Bacc("TRN2", target_bir_lowering=False, debug=False,
                   num_devices=NCORES)

    feats_sm = nc.dram_tensor("feats_sm", [T, s_steps, BL], F32, kind="ExternalInput")
    trans = nc.dram_tensor("trans", [T, T], F32, kind="ExternalInput")
    tags_bm = nc.dram_tensor("tags_bm", [BL, s_steps], I32, kind="ExternalInput")
    mask_bm = nc.dram_tensor("mask_bm", [BL, s_steps], I32, kind="ExternalInput")
    nj = (BL * s_steps) // 128
    ohprev = nc.dram_tensor("ohprev", [128, nj * T], F8, kind="ExternalInput")
    ohcur = nc.dram_tensor("ohcur", [128, nj * T], F8, kind="ExternalInput")
    nsl = s_steps * BL
    ohemA_d = nc.dram_tensor("ohemA", [128, nsl], F8, kind="ExternalInput")
    ohemB_d = nc.dram_tensor("ohemB", [128, nsl], F8, kind="ExternalInput")
    # host-precomputed index metadata (like prevT in v1): end-tag ids and
    # stop-row gather offsets (len_b-1)*BL + b
    endid_d = nc.dram_tensor("endid", [BL, 1], I32, kind="ExternalInput")
    hoff_d = nc.dram_tensor("hoff", [BL, 1], I32, kind="ExternalInput")
    out = nc.dram_tensor("out", [1, 1], F32, kind="ExternalOutput")

    h_d = nc.dram_tensor("h_d", [s_steps * BL, 1], F32)   # stop-row history (t,b)

    nch = (s_steps + chunk - 1) // chunk
    cw = chunk * BL                     # free width of one half-chunk

    with tile.TileContext(nc) as tc:
        with (
            tc.tile_pool(name="persist", bufs=1) as pp,
            tc.tile_pool(name="raw", bufs=6) as rawp,
            tc.tile_pool(name="scr", bufs=2) as scrp,
            tc.tile_pool(name="w", bufs=2) as wp,
            tc.tile_pool(name="small", bufs=1) as sp,
            tc.tile_pool(name="ps_strip", bufs=strip_bufs, space="PSUM") as ps_strip,
            tc.tile_pool(name="ps_g", bufs=1, space="PSUM") as ps_g,
            tc.tile_pool(name="ps_fin", bufs=1, space="PSUM") as ps_fin,
        ):
            # ---------------- feats chunk 0/1 DMA first (gates the scan) ----
            raws = {}

            def unit_dma(c, half):
                t0 = c * chunk
                t1c = min(s_steps, t0 + chunk)
                w = (t1c - t0) * BL
                raw = rawp.tile([128, cw], F32, tag="raw")
                nc.sync.dma_start(
                    out=raw[:, 0:w],
                    in_=feats_sm[half * 128:(half + 1) * 128, t0:t1c, :]
                    .rearrange("p a b -> p (a b)"))
                raws[(c, half)] = raw

            # mini-DMA for steps 0..7 only: the scan can start as soon as
            # these ~64KB land, instead of waiting for the full first chunk
            minis = []
            for half in (0, 1):
                rm = rawp.tile([128, 8 * BL], F32, tag="rawm")
                nc.sync.dma_start(
                    out=rm[:],
                    in_=feats_sm[half * 128:(half + 1) * 128, 0:8, :]
                    .rearrange("p a b -> p (a b)"))
                minis.append(rm)
            for half in (0, 1):
                unit_dma(0, half)

            # ---------------- constants / transition prep ----------------
            trA = pp.tile([128, T], F32, tag="trA")
            trB = pp.tile([128, T], F32, tag="trB")
            nc.sync.dma_start(out=trA[:], in_=trans[0:128, :])
            nc.sync.dma_start(out=trB[:], in_=trans[128:256, :])
            WD = F8 if f8 else BF16
            EA = pp.tile([128, T], WD, tag="EA")
            EB = pp.tile([128, T], WD, tag="EB")
            biasK = sp.tile([128, 1], F32, tag="biasK")
            nc.gpsimd.memset(biasK[:], -KAPPA)
            if f8:
                nc.scalar.activation(EA[:], trA[:], AF.Exp)
                nc.scalar.activation(EB[:], trB[:], AF.Exp)
            else:
                nc.scalar.activation(EA[:], trA[:], AF.Exp, bias=biasK[:])
                nc.scalar.activation(EB[:], trB[:], AF.Exp, bias=biasK[:])

            stA = sp.tile([128, 1], F32, tag="stA")
            stB = sp.tile([128, 1], F32, tag="stB")
            nc.sync.dma_start(out=stA[:], in_=trans[START:START + 1, 0:128])
            nc.sync.dma_start(out=stB[:], in_=trans[START:START + 1, 128:256])
            estA = sp.tile([128, 1], F32, tag="estA")
            estB = sp.tile([128, 1], F32, tag="estB")
            nc.scalar.activation(estA[:], stA[:], AF.Exp)
            nc.scalar.activation(estB[:], stB[:], AF.Exp)

            # ---------------- tags / mask prep ----------------
            mk = sp.tile([BL, s_steps], I32, tag="mk")
            nc.sync.dma_start(out=mk[:], in_=mask_bm[:])
            maskf = sp.tile([BL, s_steps], F32, tag="maskf")
            nc.vector.tensor_copy(maskf[:], mk[:])
            endid_t = sp.tile([BL, 1], I32, tag="endid_t")
            nc.sync.dma_start(out=endid_t[:], in_=endid_d[:])
            hoff_t = sp.tile([BL, 1], I32, tag="hoff_t")
            nc.sync.dma_start(out=hoff_t[:], in_=hoff_d[:])

            unit_dma(1, 0)
            unit_dma(1, 1)

            # one-hot tiles (host-encoded fp8)
            ohpv_s = pp.tile([128, nj * T], F8, tag="ohpv_s")
            ohc_s = pp.tile([128, nj * T], F8, tag="ohc_s")
            ohemA = pp.tile([128, nsl], F8, tag="ohemA")
            ohemB = pp.tile([128, nsl], F8, tag="ohemB")

            # ---------------- streaming units (emitted interleaved) --------
            # emissions: (p, t, g, h, w): chain g's step slice is flat [128, 2*bw]
            bw = BL // phase
            gw2 = 2 * bw
            expAB = pp.tile([128, 2 * nsl], BF16, tag="expAB")
            expABv = expAB[:].rearrange("p (t g h w) -> p t g h w",
                                        g=phase, h=2, w=bw)
            ep = sp.tile([128, 8 * nch], F32, tag="ep")   # emission-gold partials

            def unit_exp(c, half):
                t0 = c * chunk
                t1c = min(s_steps, t0 + chunk)
                raw = raws[(c, half)]
                lo = 0
                if c == 0:
                    lo = 8 * BL          # steps 0..7 come from the mini tiles
                    t0 = 8
                w = t1c * BL - c * chunk * BL - lo
                nc.scalar.activation(
                    expABv[:, t0:t1c, :, half, :],
                    raw[:, lo:lo + w].rearrange("p (a g v) -> p a g v",
                                                g=phase, v=bw),
                    AF.Exp, bias=biasK[:])

            def unit_exp_mini(half):
                nc.scalar.activation(
                    expABv[:, 0:8, :, half, :],
                    minis[half][:].rearrange("p (a g v) -> p a g v",
                                             g=phase, v=bw),
                    AF.Exp, bias=biasK[:])

            def unit_gold_emit(c, half, stage):
                # emission-gold dot: small DVE mult slices + one reduce
                t0 = c * chunk
                w = (min(s_steps, t0 + chunk) - t0) * BL
                ohem = ohemA if half == 0 else ohemB
                raw = raws[(c, half)]
                key = (c, half)
                if stage in (0, 1):
                    hw_ = w // 2
                    lo = stage * hw_
                    if stage == 0:
                        scr = scrp.tile([128, cw], F32, name=f"sc{key}", tag="scr")
                        raws[("scr", c, half)] = scr
                    scr = raws[("scr", c, half)]
                    eng = nc.gpsimd if pool_mult else nc.vector
                    eng.tensor_tensor(
                        scr[:, lo:lo + hw_], raw[:, lo:lo + hw_],
                        ohem[:, t0 * BL + lo:t0 * BL + lo + hw_], OP.mult)
                else:
                    # stage 2..5: quarter reduces (keep scan-DVE gaps small)
                    scr = raws[("scr", c, half)]
                    q = stage - 2
                    qw = w // 4
                    col = 8 * c + 4 * half + q
                    nc.vector.reduce_sum(ep[:, col:col + 1],
                                         scr[:, q * qw:(q + 1) * qw],
                                         axis=mybir.AxisListType.X)

            # ---------------- gold pair matmuls ----------------
            Gb = ps_g.tile([128, 2 * T], F32, tag="Gb")
            G0 = Gb[:, 0:T]
            G1 = Gb[:, T:2 * T]

            def unit_gold_pair(j, which=None):
                pv = ohpv_s[:, j * T:(j + 1) * T]
                cu = ohc_s[:, j * T:(j + 1) * T]
                if which in (None, 0):
                    nc.tensor.matmul(G0, lhsT=pv[:, 0:128], rhs=cu,
                                     start=(j == 0), stop=(j == nj - 1),
                                     skip_group_check=True)
                if which in (None, 1):
                    nc.tensor.matmul(G1, lhsT=pv[:, 128:256], rhs=cu,
                                     start=(j == 0), stop=(j == nj - 1),
                                     skip_group_check=True)

            ro = {}

            def unit_readout_prep(stage):
                if stage == 0:
                    Lb = sp.tile([BL, 1], F32, tag="Lb")
                    nc.vector.reduce_sum(Lb[:], maskf[:], axis=mybir.AxisListType.X)
                    ro["Lb"] = Lb
                else:
                    endrows = sp.tile([BL, T], F32, tag="endrows")
                    nc.gpsimd.indirect_dma_start(
                        out=endrows[:], out_offset=None, in_=trans[:],
                        in_offset=bass.IndirectOffsetOnAxis(ap=ro["endid"][:, 0:1], axis=0))
                    ro["endrows"] = endrows

            def unit_gold_post(stage):
                if stage == 0:
                    gscr = sp.tile([128, T], F32, tag="gscr")
                    tg0 = sp.tile([128, 1], F32, tag="tg0")
                    nc.vector.tensor_tensor(gscr[:], G0, trA[:], OP.mult)
                    nc.vector.reduce_sum(tg0[:], gscr[:], axis=mybir.AxisListType.X)
                    ro["tg0"] = tg0
                elif stage == 1:
                    gscr2 = sp.tile([128, T], F32, tag="gscr2")
                    tg1 = sp.tile([128, 1], F32, tag="tg1")
                    nc.vector.tensor_tensor(gscr2[:], G1, trB[:], OP.mult)
                    nc.vector.reduce_sum(tg1[:], gscr2[:], axis=mybir.AxisListType.X)
                    ro["tg1"] = tg1
                else:
                    ones128 = sp.tile([128, 1], F32, tag="ones128")
                    nc.gpsimd.memset(ones128[:], 1.0)
                    r128 = sp.tile([128, 3], F32, tag="r128")
                    nc.vector.reduce_sum(r128[:, 0:1], ep[:], axis=mybir.AxisListType.X)
                    nc.vector.tensor_copy(r128[:, 1:2], ro["tg0"][:])
                    nc.vector.tensor_copy(r128[:, 2:3], ro["tg1"][:])
                    p128 = ps_fin.tile([1, 3], F32, tag="p128")
                    nc.tensor.matmul(p128[:], lhsT=ones128[:, 0:1], rhs=r128[:],
                                     start=True, stop=True)
                    ro["p128"] = p128

            def unit_oh_dma(which):
                if which == 0:
                    nc.sync.dma_start(out=ohemA[:], in_=ohemA_d[:])
                elif which == 1:
                    nc.sync.dma_start(out=ohemB[:], in_=ohemB_d[:])
                elif which == 2:
                    nc.sync.dma_start(out=ohpv_s[:], in_=ohprev[:])
                else:
                    nc.sync.dma_start(out=ohc_s[:], in_=ohcur[:])

            # ---------------- schedule ----------------
            for half in (0, 1):
                unit_exp_mini(half)

            sched = {t: [] for t in range(1, s_steps + 1)}

            def at(t, fn, *a):
                sched[min(s_steps, max(1, t))].append((fn, a))

            ro["endid"] = endid_t
            ro["hoff"] = hoff_t
            at(2, unit_exp, 0, 0)
            at(3, unit_exp, 0, 1)
            for i in range(4):
                at(4 + i, unit_oh_dma, i)
            for st in range(2):
                at(100 + 6 * st, unit_readout_prep, st)
            for c in range(1, nch):
                base = (c - 1) * chunk
                for half in (0, 1):
                    if c + 1 < nch:
                        at(base + 8 + half, unit_dma, c + 1, half)
                    at(base + 16 + 8 * half, unit_exp, c, half)
            # emission-gold DVE slices for chunk c run while chunk c+1 scans
            for c in range(nch):
                base = c * chunk + chunk // 2
                for half in (0, 1):
                    at(base + 4 + 24 * half, unit_gold_emit, c, half, 0)
                    at(base + 8 + 24 * half, unit_gold_emit, c, half, 1)
                    for q in range(4):
                        at(base + 12 + 24 * half + 3 * q, unit_gold_emit, c, half, 2 + q)
            if gold_inline:
                lead = s_steps // 4
                spacing = max(1, (s_steps - lead - 24) // nj)
                for j in range(nj):
                    # split the pair: one 256-col matmul per inter-step slot
                    at(lead + j * spacing, unit_gold_pair, j, 0)
                    at(lead + j * spacing + 2, unit_gold_pair, j, 1)

            # ---------------- the scan ----------------
            H = pp.tile([128, nsl], F32, tag="H")

            ws = []
            for g in range(phase):
                w0 = wp.tile([128, gw2], BF16, name=f"w{g}", tag=f"w{g}")
                nc.vector.tensor_scalar(w0[:, 0:bw], expABv[:, 0, g, 0, :],
                                        estA[:, 0:1], None, OP.mult)
                nc.vector.tensor_scalar(w0[:, bw:gw2], expABv[:, 0, g, 1, :],
                                        estB[:, 0:1], None, OP.mult)
                ws.append(w0)

            strips = [None] * phase
            for t in range(1, s_steps + 1):
                for fn, a in sched[t]:
                    fn(*a)
                j = (t - 1) % kcap
                for g in range(phase):
                    if j == 0:
                        strips[g] = ps_strip.tile([128, kcap * gw2], F32,
                                                  name=f"st{g}", tag=f"st{g}")
                    strip = strips[g]
                    sl = strip[:, j * gw2:(j + 1) * gw2]
                    wg = ws[g]
                    nc.tensor.matmul(sl[:, 0:bw], lhsT=EA[:, 0:128], rhs=wg[:, 0:bw],
                                     start=True, stop=False, skip_group_check=True)
                    nc.tensor.matmul(sl[:, 0:bw], lhsT=EB[:, 0:128], rhs=wg[:, bw:gw2],
                                     start=False, stop=True, skip_group_check=True)
                    nc.tensor.matmul(sl[:, bw:gw2], lhsT=EA[:, 128:256], rhs=wg[:, 0:bw],
                                     start=True, stop=False, skip_group_check=True)
                    nc.tensor.matmul(sl[:, bw:gw2], lhsT=EB[:, 128:256], rhs=wg[:, bw:gw2],
                                     start=False, stop=True, skip_group_check=True)
                    if t < s_steps:
                        wg2t = wp.tile([128, gw2], BF16, name=f"w{g}", tag=f"w{g}")
                        off = (t * phase + g) * gw2
                        nc.vector.tensor_tensor(
                            wg2t[:], sl[:], expAB[:, off:off + gw2], OP.mult)
                        ws[g] = wg2t
                    if j == kcap - 1 or t == s_steps:
                        tlo = t - 1 - j
                        stv = strip[96:128, 0:(j + 1) * gw2].rearrange(
                            "p (k h w) -> p k h w", h=2, w=bw)
                        hv = H[96:128, tlo * BL:t * BL].rearrange(
                            "p (k b) -> p k b", b=BL)
                        nc.scalar.copy(hv[:, :, g * bw:(g + 1) * bw],
                                       stv[:, :, 1, :])
                        if g == phase - 1:
                            nc.sync.dma_start(
                                out=h_d[tlo * BL:t * BL, 0],
                                in_=H[127:128, tlo * BL:t * BL])

            # ---------------- post-loop ----------------
            # gold-matrix contraction AFTER the scan: scheduling it in-scan
            # raced the last emission-gold units' ep writes (Tile missed the
            # overlapping-slice dependency) and made results nondeterministic
            for st in range(3):
                unit_gold_post(st)
            hstop = sp.tile([BL, 1], F32, tag="hstop")
            nc.gpsimd.indirect_dma_start(
                out=hstop[:], out_offset=None, in_=h_d[:],
                in_offset=bass.IndirectOffsetOnAxis(ap=ro["hoff"][:, 0:1], axis=0))

            lnh = sp.tile([BL, 1], F32, tag="lnh")
            nc.scalar.activation(lnh[:], hstop[:], AF.Ln)
            fwdb = sp.tile([BL, 1], F32, tag="fwdb")
            kl = sp.tile([BL, 1], F32, tag="kl")
            nc.vector.tensor_scalar(kl[:], ro["Lb"][:], KAPPA, None, OP.mult)
            nc.vector.tensor_tensor(fwdb[:], kl[:], lnh[:], OP.add)

            # gold matrix already contracted in-scan (unit_gold_post)
            p128 = ro["p128"]

            r16 = sp.tile([BL, 2], F32, tag="r16")
            nc.vector.tensor_copy(r16[:, 0:1], fwdb[:])
            nc.vector.tensor_copy(r16[:, 1:2], ro["endrows"][:, STOP:STOP + 1])
            ones16 = sp.tile([BL, 1], F32, tag="ones16")
            nc.gpsimd.memset(ones16[:], 1.0)
            p16 = ps_fin.tile([1, 2], F32, tag="p16")
            nc.tensor.matmul(p16[:], lhsT=ones16[:, 0:1], rhs=r16[:], start=True, stop=True)

            s128 = sp.tile([1, 3], F32, tag="s128")
            s16 = sp.tile([1, 2], F32, tag="s16")
            nc.vector.tensor_copy(s128[:], p128[:])
            nc.vector.tensor_copy(s16[:], p16[:])
            gold128 = sp.tile([1, 1], F32, tag="gold128")
            nc.vector.reduce_sum(gold128[:], s128[:], axis=mybir.AxisListType.X)
            fin0 = sp.tile([1, 1], F32, tag="fin0")
            nc.vector.tensor_tensor(fin0[:], s16[:, 0:1], s16[:, 1:2], OP.subtract)
            fin = sp.tile([1, 1], F32, tag="fin")
            nc.vector.tensor_tensor(fin[:], fin0[:], gold128[:], OP.subtract)
            nc.sync.dma_start(out=out[:], in_=fin[:])

    if do_compile:
        nc.compile()
    return nc


def make_in_maps(feats, transitions, tags, mask, s_steps=S):
    feats = np.asarray(feats, dtype=np.float32)
    transitions = np.asarray(transitions, dtype=np.float32)
    tags = np.asarray(tags).astype(np.int32)
    mask = np.asarray(mask).astype(np.int32)
    nj = (BL * s_steps) // 128
    nsl = s_steps * BL
    eye = np.eye(T, dtype=F8NP)
    zrow = np.zeros((1, T), dtype=F8NP)
    ohtab = np.concatenate([eye, zrow], axis=0)
    in_maps = []
    for c in range(NCORES):
        bs = slice(c * BL, (c + 1) * BL)
        f = np.ascontiguousarray(feats[bs, :s_steps, :].transpose(2, 1, 0))
        tg = np.ascontiguousarray(tags[bs, :s_steps])
        mk = np.ascontiguousarray(mask[bs, :s_steps])
        pv = np.concatenate(
            [np.full((BL, 1), START, np.int32), tg[:, :-1]], axis=1)
        tgm = np.where(mk > 0, tg, T)
        pvm = np.where(mk > 0, pv, T)

        def btT(x):
            return x.reshape(-1).reshape(-1, 128).T

        ohc = np.ascontiguousarray(ohtab[btT(tgm)].reshape(128, nj * T))
        ohp = np.ascontiguousarray(ohtab[btT(pvm)].reshape(128, nj * T))
        # emission one-hots: [state(128) x (t,b)] per state-half
        tb = np.ascontiguousarray(tgm.T).reshape(nsl)      # (t, b) order
        ohem = np.ascontiguousarray(ohtab[tb][:, :].T)     # [T, nsl]
        # index metadata: end-tag ids and stop-row gather offsets
        L = mk.sum(axis=1)
        endid = np.take_along_axis(tg, (L - 1)[:, None], axis=1).astype(np.int32)
        hoff = ((L - 1) * BL + np.arange(BL)).astype(np.int32)[:, None]
        in_maps.append({
            "feats_sm": f,
            "trans": transitions,
            "tags_bm": tg,
            "mask_bm": mk,
            "ohprev": ohp,
            "ohcur": ohc,
            "ohemA": np.ascontiguousarray(ohem[0:128]),
            "ohemB": np.ascontiguousarray(ohem[128:256]),
            "endid": endid,
            "hoff": hoff,
        })
    return in_maps


_CACHE = {}


def kernel(**inputs):
    if "nc" not in _CACHE:
        _CACHE["nc"] = build_program()
    nc = _CACHE["nc"]
    in_maps = make_in_maps(inputs["feats"], inputs["transitions"],
                           inputs["tags"], inputs["mask"])
    res = run_bass_kernel_spmd(nc, in_maps, core_ids=list(range(NCORES)))
    total = np.float64(0.0)
    for r in res.results:
        total += np.float64(r["out"].reshape(()))
    return np.asarray(total, dtype=np.float32).reshape(())
